# revision 66
# baseline (speedup 1.0000x reference)
"""LoRA attention Bass kernel for 8x Trainium2 NeuronCores (bf16, pipelined).

Sharding (Megatron tensor-parallel over heads):
  - Each of the 8 cores owns 2 heads (128 projection columns).
  - q/k/v projections column-sharded; out projection row-sharded;
    per-core partial outputs are summed on the host.
  - LoRA is merged into the base weights on the host (w_eff = w + a@u*scaling),
    which is exact up to rounding.

Device kernel (per core), all matmuls in bf16 (fp32 PSUM accumulation):
  - Projections: qT/kT computed transposed ([proj_col, seq]) from xT tiles;
    v in natural [seq, col] layout from the same tiles.  Emitted in 256-wide
    half-chunks so q|k and v share one 2-bank PSUM slot.
  - Attention per (batch, 512-wide s-chunk): S^T = K @ Q^T per head into a
    shared [128, 2, 512] PSUM tile, ONE exp over both heads ([128,1024] ACT
    op), P@V with lhsT=[v | ones] so the softmax denominator falls out of the
    same matmul.  The PV matmuls trail the score matmuls by three t-steps so
    the PE never blocks on the ACT engine.
  - Normalization: reciprocal_approx_fast on the denominator row, K=1 matmul
    broadcast, fused out-projection, DMA to DRAM.
  - Software pipelining: the normalize+out-proj tail of chunk i and the
    projection work for batch b+1 are chopped into "pieces" and interleaved
    into chunk i+1's t-loop, filling PE idle slots while ACT (exp) runs
    back-to-back.  ACT is the critical path at ~1.06us per 1024-wide exp.

PSUM budget (8 banks): scores 2 slots x 2 banks + pv A/B 1 bank each +
shared proj/bc/out-proj slot 2 banks.
"""

import numpy as np
import ml_dtypes

import concourse.bass as bass
import concourse.mybir as mybir
import concourse.tile as tile
from concourse import bacc
from concourse.bass_utils import run_bass_kernel_spmd

F32 = mybir.dt.float32
F32R = mybir.dt.float32r
BF16 = mybir.dt.bfloat16
AF = mybir.ActivationFunctionType
MUL = mybir.AluOpType.mult

BF = ml_dtypes.bfloat16

N_CORES = 8

# Full-problem dims (hardcoded per spec)
D_MODEL = 1024
N_HEADS = 16
D_K = 64
LORA_R = 8
SCALING = 2.0
B = 4
S = 2048


class Cfg:
    """Kernel build configuration (parameterized so tests can build small)."""

    def __init__(self, b=B, s=S, d=D_MODEL, cpc=128, dk=D_K,
                 serial=False, debug=False):
        self.serial = serial            # no piece interleaving (bisect aid)
        self.debug = debug              # dump qT/kT/v to DRAM
        self.b = b                      # batches
        self.s = s                      # seq per batch
        self.d = d                      # model dim (projection contraction)
        self.cpc = cpc                  # projection cols per core (2 heads x 64)
        self.dk = dk                    # head dim
        self.seq = b * s                # total rows
        self.nkc = d // 128             # contraction chunks for projections
        self.sc = min(512, s)           # attention s-chunk width
        self.hw = self.sc // 2          # projection half-chunk width
        self.nsb = s // self.sc         # s-chunks per batch
        self.nt = s // 128              # t-chunks per batch
        self.nj = self.sc // 128        # 128-row groups per s-chunk
        self.ntt = self.seq // 128      # t-chunks total
        self.ne = max(1, d // self.sc)  # out-proj column groups
        assert self.ne <= 2 and self.hw % 128 == 0


def _build_nc(cfg: Cfg):
    c = cfg
    dk = c.dk
    nc = bacc.Bacc("TRN2", target_bir_lowering=False, debug=False,
                   num_devices=N_CORES)

    xT = nc.dram_tensor("xT", [c.d, c.seq], BF16, kind="ExternalInput").ap()
    wq = nc.dram_tensor("wq", [c.d, c.cpc], BF16, kind="ExternalInput").ap()
    wk = nc.dram_tensor("wk", [c.d, c.cpc], BF16, kind="ExternalInput").ap()
    wv = nc.dram_tensor("wv", [c.d, c.cpc], BF16, kind="ExternalInput").ap()
    wo = nc.dram_tensor("wo", [c.cpc, c.d], BF16, kind="ExternalInput").ap()
    bq = nc.dram_tensor("bq", [c.cpc, 1], F32, kind="ExternalInput").ap()
    bk = nc.dram_tensor("bk", [c.cpc, 1], F32, kind="ExternalInput").ap()
    ident = nc.dram_tensor("ident", [128, 128], BF16, kind="ExternalInput").ap()
    out = nc.dram_tensor("out", [c.seq, c.d], F32, kind="ExternalOutput").ap()
    if c.debug:
        qTd = nc.dram_tensor("qTd", [128, c.seq], BF16, kind="ExternalOutput").ap()
        kTd = nc.dram_tensor("kTd", [128, c.seq], BF16, kind="ExternalOutput").ap()
        vd = nc.dram_tensor("vd", [128, c.ntt, 2 * c.dk + 2], BF16,
                            kind="ExternalOutput").ap()
        scd = nc.dram_tensor("scd", [128, 2, c.sc], F32, kind="ExternalOutput").ap()
        ed = nc.dram_tensor("ed", [128, 2, c.sc], BF16, kind="ExternalOutput").ap()
        pvd = nc.dram_tensor("pvd", [c.dk + 1, 2, c.sc], F32,
                             kind="ExternalOutput").ap()
        recd = nc.dram_tensor("recd", [1, 2, c.sc], F32, kind="ExternalOutput").ap()
        normd = nc.dram_tensor("normd", [c.dk, 2, c.sc], BF16,
                               kind="ExternalOutput").ap()

    BK = 2048  # PSUM bank bytes

    def group_flags(items):
        """items: [(key, byte_offset)] of matmul writes into one PSUM slot.
        PSUM accumulation groups are per 2KB bank: return key -> (is_first,
        is_last) touch of its bank so start/stop is set exactly once per
        bank."""
        first, last = {}, {}
        for key, off in items:
            b = off // BK
            if b not in first:
                first[b] = key
            last[b] = key
        return {key: (first[off // BK] == key, last[off // BK] == key)
                for key, off in items}

    with tile.TileContext(nc) as tc:
        with tc.tile_pool(name="persist", bufs=1) as persist:
            qT_sb = persist.tile([128, c.seq], BF16, tag="qT")
            kT_sb = persist.tile([128, c.seq], BF16, tag="kT")
            # v natural + ones cols: [0:dk]=headA, dk=ones, [dk+1:2dk+1]=headB,
            # 2dk+1=ones
            v_sb = persist.tile([128, c.ntt, 2 * dk + 2], BF16, tag="v")
            wq_sb = persist.tile([128, c.nkc, c.cpc], BF16, tag="wq")
            wk_sb = persist.tile([128, c.nkc, c.cpc], BF16, tag="wk")
            wv_sb = persist.tile([128, c.nkc, c.cpc], BF16, tag="wv")
            wo_sb = persist.tile([c.cpc, c.d], BF16, tag="wo")
            bq_sb = persist.tile([c.cpc, 1], F32, tag="bq")
            bk_sb = persist.tile([c.cpc, 1], F32, tag="bk")
            id_sb = persist.tile([128, 128], BF16, tag="ident")
            nc.sync.dma_start(out=id_sb[:], in_=ident)

            nc.sync.dma_start(out=wq_sb[:], in_=wq.rearrange("(kc p) m -> p kc m", p=128))
            nc.sync.dma_start(out=wk_sb[:], in_=wk.rearrange("(kc p) m -> p kc m", p=128))
            nc.sync.dma_start(out=wv_sb[:], in_=wv.rearrange("(kc p) m -> p kc m", p=128))
            nc.sync.dma_start(out=wo_sb[:], in_=wo[:])
            nc.sync.dma_start(out=bq_sb[:], in_=bq[:])
            nc.sync.dma_start(out=bk_sb[:], in_=bk[:])

            ones_f32 = persist.tile([128, 1], F32, tag="ones_f32")
            nc.vector.memset(ones_f32[:], 1.0)
            # ones columns for the fused softmax denominator
            nc.vector.tensor_copy(
                v_sb[:, :, dk:dk + 1],
                ones_f32[:].unsqueeze(1).to_broadcast([128, c.ntt, 1]))
            nc.vector.tensor_copy(
                v_sb[:, :, 2 * dk + 1:2 * dk + 2],
                ones_f32[:].unsqueeze(1).to_broadcast([128, c.ntt, 1]))
            # ones row at partition dk (bf16, for the bf16 bc matmul)
            ones_bc = persist.tile([dk + 1, dk], BF16, tag="ones_bc")
            nc.vector.tensor_copy(
                ones_bc[:], ones_f32[0:dk + 1, :].to_broadcast([dk + 1, dk]))
            # zero-weights row + defined rhs row for PSUM bank-clearing
            # matmuls: on HW has_written bits persist across accumulation
            # groups, so every group must open with a start=True matmul that
            # covers the ENTIRE bank it accumulates into.
            zrow_sb = persist.tile([1, 128], BF16, tag="zrow")
            nc.vector.memset(zrow_sb[:], 0.0)
            orow_sb = persist.tile([1, 512], BF16, tag="orow")
            nc.vector.memset(orow_sb[:], 1.0)

            with tc.tile_pool(name="xp", bufs=3) as xpool, \
                 tc.tile_pool(name="ep", bufs=7) as epool, \
                 tc.tile_pool(name="np", bufs=2) as npool, \
                 tc.tile_pool(name="rp", bufs=2) as rpool, \
                 tc.tile_pool(name="bp", bufs=2) as bpool, \
                 tc.tile_pool(name="op", bufs=3) as opool, \
                 tc.tile_pool(name="scp", bufs=2, space="PSUM") as scp, \
                 tc.tile_pool(name="pvp", bufs=1, space="PSUM") as pvp, \
                 tc.tile_pool(name="wkp", bufs=1, space="PSUM") as wkp:

                def work_tile(name):
                    # shared 2-bank PSUM slot for proj / bc / out-proj
                    return wkp.tile([128, 2, c.sc], F32, tag="wk", name=name)

                def proj_pieces(b, pc):
                    """Piece closures computing q/k/vT for s-chunk pc of
                    batch b with full-width (512) matmuls: q -> work bank0,
                    k -> bank1, then vT reuses the slot; vT is converted to
                    natural layout by PE transposes (identity rhs)."""
                    pieces = []
                    s0 = b * c.s + pc * c.sc
                    tc0 = s0 // 128
                    qfl = group_flags([("q", 0), ("k", c.sc * 4)])
                    PF = 4  # x-tile DMA prefetch depth (pieces ahead)
                    xts = {}
                    st8 = {}

                    def fetch_x(kc):
                        x_t = xpool.tile([128, c.sc], BF16, tag="xf", bufs=8,
                                         name=f"x_{b}_{pc}_{kc}")
                        nc.sync.dma_start(
                            out=x_t[:],
                            in_=xT[kc * 128:(kc + 1) * 128, s0:s0 + c.sc])
                        xts[kc] = x_t

                    for kc in range(c.nkc):
                        def fq(kc=kc):
                            if kc == 0:
                                st8["qk"] = work_tile(f"pqk_{b}_{pc}")
                                for p in range(min(PF + 1, c.nkc)):
                                    fetch_x(p)
                            elif kc + PF < c.nkc:
                                fetch_x(kc + PF)
                            qk = st8["qk"]
                            nc.tensor.matmul(qk[:, 0, :], wq_sb[:, kc, :],
                                             xts[kc][:],
                                             start=(kc == 0) and qfl["q"][0],
                                             stop=(kc == c.nkc - 1) and qfl["q"][1])
                        pieces.append(fq)
                    for kc in range(c.nkc):
                        def fk(kc=kc):
                            qk = st8["qk"]
                            nc.tensor.matmul(qk[:, 1, :], wk_sb[:, kc, :],
                                             xts[kc][:],
                                             start=(kc == 0) and qfl["k"][0],
                                             stop=(kc == c.nkc - 1) and qfl["k"][1])
                            if kc == c.nkc - 1:
                                nc.vector.tensor_scalar_add(
                                    qT_sb[:, s0:s0 + c.sc], qk[:, 0, :],
                                    bq_sb[:])
                                nc.vector.tensor_scalar_add(
                                    kT_sb[:, s0:s0 + c.sc], qk[:, 1, :],
                                    bk_sb[:])
                        pieces.append(fk)
                    for kc in range(c.nkc):
                        def fv(kc=kc):
                            if kc == 0:
                                st8["vt"] = work_tile(f"pvt_{b}_{pc}")
                            vt = st8["vt"]
                            nc.tensor.matmul(vt[:, 0, :], wv_sb[:, kc, :],
                                             xts[kc][:],
                                             start=(kc == 0),
                                             stop=(kc == c.nkc - 1))
                            if kc == c.nkc - 1:
                                vts = bpool.tile([128, c.sc], BF16, tag="vts",
                                                 name=f"vts_{b}_{pc}")
                                nc.vector.tensor_copy(vts[:], vt[:, 0, :])
                                st8["vts"] = vts
                        pieces.append(fv)
                    for j in range(c.nj):
                        def ftr(j=j):
                            vts = st8["vts"]
                            tp = work_tile(f"vtr_{b}_{pc}_{j}")
                            tb = tp[:, 1, :].bitcast(BF16)
                            nc.tensor.matmul(tb[:, 0:128],
                                             vts[:, j * 128:(j + 1) * 128],
                                             id_sb[:], is_transpose=True,
                                             start=True, stop=True)
                            dst = v_sb[:, tc0 + j, :].rearrange(
                                "p (h x) -> p h x", x=dk + 1)[:, :, 0:dk]
                            src = tb[:, 0:128].rearrange(
                                "p (h x) -> p h x", x=dk)
                            nc.vector.tensor_copy(dst, src)
                        pieces.append(ftr)
                    return pieces

                def emit_proj_full(b, pc):
                    """Prologue-only projection: full 512-wide q/k matmuls
                    using a free scores-pool slot (t-loops haven't started)."""
                    s0 = b * c.s + pc * c.sc
                    tc0 = s0 // 128
                    qk = scp.tile([128, 2, c.sc], F32, tag="sc",
                                  name=f"pqk_{b}_{pc}")
                    vw = work_tile(f"pv_{b}_{pc}")
                    nc.tensor.matmul(vw[:, 1, :], zrow_sb[:], orow_sb[:, 0:c.sc],
                                     start=True, stop=False)
                    xts = {}

                    def fetch(kc):
                        x_t = xpool.tile([128, c.sc], BF16, tag="xf",
                                         name=f"xf_{b}_{pc}_{kc}", bufs=8)
                        nc.sync.dma_start(
                            out=x_t[:],
                            in_=xT[kc * 128:(kc + 1) * 128, s0:s0 + c.sc])
                        xts[kc] = x_t

                    for p in range(min(6, c.nkc)):
                        fetch(p)
                    for kc in range(c.nkc):
                        if kc + 6 < c.nkc:
                            fetch(kc + 6)
                        x_t = xts.pop(kc)
                        k0, kN = kc == 0, kc == c.nkc - 1
                        qfl = group_flags([("q", 0), ("k", c.sc * 4)])
                        nc.tensor.matmul(qk[:, 0, :], wq_sb[:, kc, :], x_t[:],
                                         start=k0 and qfl["q"][0],
                                         stop=kN and qfl["q"][1])
                        nc.tensor.matmul(qk[:, 1, :], wk_sb[:, kc, :], x_t[:],
                                         start=k0 and qfl["k"][0],
                                         stop=kN and qfl["k"][1])
                        for j in range(c.nj):
                            nc.tensor.matmul(vw[:, 1, j * 128:(j + 1) * 128],
                                             x_t[:, j * 128:(j + 1) * 128],
                                             wv_sb[:, kc, :],
                                             start=False, stop=kN and j == c.nj - 1)
                    nc.vector.tensor_scalar_add(qT_sb[:, s0:s0 + c.sc],
                                                qk[:, 0, :], bq_sb[:])
                    nc.vector.tensor_scalar_add(kT_sb[:, s0:s0 + c.sc],
                                                qk[:, 1, :], bk_sb[:])
                    v4 = vw[:, 1, :].rearrange("p (j x) -> p j x", x=128)
                    nc.vector.tensor_copy(v_sb[:, tc0:tc0 + c.nj, 0:dk],
                                          v4[:, :, 0:dk])
                    nc.vector.tensor_copy(
                        v_sb[:, tc0:tc0 + c.nj, dk + 1:2 * dk + 1],
                        v4[:, :, dk:2 * dk])

                def tail_pieces(b, sb, pva, pvb):
                    """Normalize + out-project chunk (b, sb); emitted one
                    chunk late so the work interleaves into the next t-loop."""
                    s0g = b * c.s + sb * c.sc
                    # Free the PV PSUM banks ASAP: copy the full accumulators
                    # to SBUF right away; everything downstream reads the
                    # copy.  (pvp bufs=1 — the next chunk's PV matmuls wait
                    # only on this copy, not on the whole normalize chain.)
                    pvs = rpool.tile([dk + 1, 2, c.sc], F32, tag="pvs",
                                     name=f"pvs_{b}_{sb}")
                    nc.vector.tensor_copy(pvs[:, 0, :], pva[:])
                    nc.vector.tensor_copy(pvs[:, 1, :], pvb[:])
                    # denominator rows as bf16 for the broadcast matmul
                    # (reciprocal_approx_fast is broken for single-row APs at
                    # partition 64 on HW, so broadcast the DENOMINATOR first,
                    # then recip on the 64-partition broadcast tile)
                    denb = rpool.tile([dk + 1, 2, c.sc], BF16, tag="denb",
                                      name=f"denb_{b}_{sb}")
                    nc.vector.tensor_copy(denb[dk:dk + 1, :, :],
                                          pvs[dk:dk + 1, :, :])
                    if c.debug and b == 0 and sb == 0:
                        nc.sync.dma_start(out=pvd, in_=pvs[:])
                    st8 = {}
                    pieces = []

                    def bc_f():
                        bfl = group_flags([("a", 0), ("b", c.sc * 4)])
                        bcp = work_tile(f"bc_{b}_{sb}")
                        nc.tensor.matmul(bcp[0:dk, 0, :],
                                         ones_bc[dk:dk + 1, :],
                                         denb[dk:dk + 1, 0, :],
                                         start=bfl["a"][0], stop=bfl["a"][1])
                        nc.tensor.matmul(bcp[0:dk, 1, :],
                                         ones_bc[dk:dk + 1, :],
                                         denb[dk:dk + 1, 1, :],
                                         start=bfl["b"][0], stop=bfl["b"][1])
                        bcs = bpool.tile([dk, 2, c.sc], F32, tag="bcs",
                                         name=f"bcs_{b}_{sb}")
                        bcr = bpool.tile([dk, 2, c.sc], F32, tag="bcr",
                                         name=f"bcr_{b}_{sb}")
                        norm = npool.tile([dk, 2, c.sc], BF16, tag="norm",
                                          name=f"norm_{b}_{sb}")
                        nc.vector.tensor_copy(bcs[:], bcp[0:dk, :, :])
                        nc.vector.reciprocal_approx_fast(out=bcr[:], in_=bcs[:])
                        nc.vector.tensor_tensor(norm[:, 0, :], pvs[0:dk, 0, :],
                                                bcr[:, 0, :], MUL)
                        nc.vector.tensor_tensor(norm[:, 1, :], pvs[0:dk, 1, :],
                                                bcr[:, 1, :], MUL)
                        if c.debug and b == 0 and sb == 0:
                            nc.sync.dma_start(out=recd, in_=bcr[0:1, :, :])
                            nc.sync.dma_start(out=normd, in_=norm[:])
                        st8["norm"] = norm
                    pieces.append(bc_f)

                    def asm_f():
                        # stack the two heads' normalized attention into one
                        # [128, sc] tile via K=64 identity matmuls (headB
                        # lands at partitions 64-127), so each out-proj group
                        # is a single K=128 matmul against unsliced w_o
                        norm = st8["norm"]
                        aps = work_tile(f"asm_{b}_{sb}")
                        nc.tensor.matmul(aps[0:dk, 0, :],
                                         id_sb[0:dk, 0:dk], norm[:, 0, :],
                                         start=True, stop=True)
                        nc.tensor.matmul(aps[dk:2 * dk, 0, :],
                                         id_sb[0:dk, 0:dk], norm[:, 1, :],
                                         start=True, stop=True)
                        n2 = npool.tile([128, c.sc], BF16, tag="n2",
                                        name=f"n2_{b}_{sb}")
                        nc.vector.tensor_copy(n2[:], aps[:, 0, :])
                        st8["n2"] = n2
                    pieces.append(asm_f)

                    for j in range(c.nj):
                        def o_f(j=j):
                            n2 = st8["n2"]
                            ops = work_tile(f"ops_{b}_{sb}_{j}")
                            for ei in range(c.ne):
                                nc.tensor.matmul(
                                    ops[:, ei, 0:min(c.sc, c.d)],
                                    n2[:, j * 128:(j + 1) * 128],
                                    wo_sb[:, ei * c.sc:min((ei + 1) * c.sc, c.d)],
                                    start=True, stop=True)
                            o_t = opool.tile([128, c.d], F32, tag="o",
                                             name=f"ot_{b}_{sb}_{j}")
                            ov = o_t.rearrange("p (e x) -> p e x",
                                               x=min(c.sc, c.d))
                            nc.vector.tensor_copy(
                                ov[:, :, :], ops[:, 0:c.ne, 0:min(c.sc, c.d)])
                            nc.sync.dma_start(
                                out=out[s0g + j * 128:s0g + (j + 1) * 128, :],
                                in_=o_t[:])
                        pieces.append(o_f)
                    return pieces

                def emit_tloop(b, sb, pieces):
                    s0 = b * c.s + sb * c.sc
                    pva = pvp.tile([dk + 1, c.sc], F32, tag="pva",
                                   name=f"pva_{b}_{sb}")
                    pvb = pvp.tile([dk + 1, c.sc], F32, tag="pvb",
                                   name=f"pvb_{b}_{sb}")
                    pend = []
                    npc = len(pieces)
                    pi = 0
                    lag = min(5, c.nt)  # pv trails exp so PE never waits ACT
                    sfl = group_flags([("a", 0), ("b", c.sc * 4)])

                    def emit_pv(pt, pe_t):
                        tci = b * c.nt + pt
                        nc.tensor.matmul(pva[:], v_sb[:, tci, 0:dk + 1],
                                         pe_t[:, 0, :], start=(pt == 0),
                                         stop=(pt == c.nt - 1))
                        nc.tensor.matmul(pvb[:],
                                         v_sb[:, tci, dk + 1:2 * dk + 2],
                                         pe_t[:, 1, :], start=(pt == 0),
                                         stop=(pt == c.nt - 1))

                    for t in range(c.nt):
                        t0 = b * c.s + t * 128
                        scs = scp.tile([128, 2, c.sc], F32, tag="sc",
                                       name=f"scs_{b}_{sb}_{t}")
                        nc.tensor.matmul(scs[:, 0, :],
                                         kT_sb[0:dk, t0:t0 + 128],
                                         qT_sb[0:dk, s0:s0 + c.sc],
                                         start=sfl["a"][0], stop=sfl["a"][1])
                        nc.tensor.matmul(scs[:, 1, :],
                                         kT_sb[dk:2 * dk, t0:t0 + 128],
                                         qT_sb[dk:2 * dk, s0:s0 + c.sc],
                                         start=sfl["b"][0], stop=sfl["b"][1])
                        e_t = epool.tile([128, 2, c.sc], BF16, tag="e",
                                         name=f"e_{b}_{sb}_{t}")
                        if c.debug and b == 0 and sb == 0 and t == 0:
                            scs_d = opool.tile([128, 2, c.sc], F32, tag="scd",
                                               name="scs_dump")
                            nc.vector.tensor_copy(scs_d[:], scs[:])
                            nc.sync.dma_start(out=scd, in_=scs_d[:])
                        nc.scalar.activation(e_t[:], scs[:], AF.Exp,
                                             scale=1.0 / np.sqrt(dk))
                        if c.debug and b == 0 and sb == 0 and t == 0:
                            nc.sync.dma_start(out=ed, in_=e_t[:])
                        want = max((npc * (t + 1)) // c.nt, min(npc, 1))
                        while pi < want:
                            pieces[pi]()
                            pi += 1
                        pend.append((t, e_t))
                        if len(pend) >= lag:
                            emit_pv(*pend.pop(0))
                    for pt, pe_t in pend:
                        emit_pv(pt, pe_t)
                    return pva, pvb

                # ---- emission: short prologue, proj queue, pipelined chunks
                # Prologue covers proj(0, 0..nsb-2) serially; the remaining
                # projection chunks stream through a queue, one per attention
                # chunk (two for chunk 0, which has no tail pieces).  Batch
                # b+1's projections all land before attention chunk (b+1, 0).
                npro = max(c.nsb - 2, 1)
                for pc in range(npro):
                    emit_proj_full(0, pc)
                pq = [(0, pc) for pc in range(npro, c.nsb)]
                pq += [(b + 1, sb) for b in range(c.b - 1)
                       for sb in range(c.nsb)]
                prev = None
                ci = 0
                for b in range(c.b):
                    for sb in range(c.nsb):
                        pieces = []
                        if prev is not None:
                            pieces += tail_pieces(*prev)
                        pops = 3 if ci == 0 else 1
                        for _ in range(pops):
                            if pq:
                                pieces += proj_pieces(*pq.pop(0))
                        if c.serial:
                            for f in pieces:
                                f()
                            pieces = []
                        pva, pvb = emit_tloop(b, sb, pieces)
                        prev = (b, sb, pva, pvb)
                        ci += 1
                for f in tail_pieces(*prev):
                    f()
                if c.debug:
                    nc.sync.dma_start(out=qTd, in_=qT_sb[:])
                    nc.sync.dma_start(out=kTd, in_=kT_sb[:])
                    nc.sync.dma_start(out=vd, in_=v_sb[:])

    nc.compile()
    return nc


_NC_CACHE = {}


def get_nc(cfg: Cfg | None = None):
    cfg = cfg or Cfg()
    key = (cfg.b, cfg.s, cfg.d, cfg.cpc, cfg.dk, cfg.serial, cfg.debug)
    if key not in _NC_CACHE:
        _NC_CACHE[key] = _build_nc(cfg)
    return _NC_CACHE[key]


def _merge(w, a, u):
    return (np.asarray(w, np.float64)
            + (np.asarray(a, np.float64) @ np.asarray(u, np.float64))
            * SCALING)


def make_in_maps(inputs, cfg: Cfg | None = None):
    """Host-side prep: merge LoRA, transpose x, cast to bf16, shard by head."""
    c = cfg or Cfg()
    x = np.asarray(inputs["x"], np.float32)
    wq_eff = _merge(inputs["w_q"], inputs["a_q"], inputs["u_q"])
    wk_eff = _merge(inputs["w_k"], inputs["a_k"], inputs["u_k"])
    wv_eff = _merge(inputs["w_v"], inputs["a_v"], inputs["u_v"])
    w_o = np.asarray(inputs["w_o"], np.float32)
    b_q = np.asarray(inputs["b_q"], np.float32)
    b_k = np.asarray(inputs["b_k"], np.float32)

    xT = np.ascontiguousarray(x.reshape(c.seq, c.d).T.astype(BF))
    ident = np.eye(128, dtype=BF)
    in_maps = []
    for i in range(N_CORES):
        sl = slice(i * c.cpc, (i + 1) * c.cpc)
        in_maps.append({
            "xT": xT,
            "ident": ident,
            "wq": np.ascontiguousarray(wq_eff[:, sl].astype(BF)),
            "wk": np.ascontiguousarray(wk_eff[:, sl].astype(BF)),
            "wv": np.ascontiguousarray(wv_eff[:, sl].astype(BF)),
            "wo": np.ascontiguousarray(w_o[sl, :].astype(BF)),
            "bq": np.ascontiguousarray(b_q[sl]).reshape(c.cpc, 1),
            "bk": np.ascontiguousarray(b_k[sl]).reshape(c.cpc, 1),
        })
    return in_maps


def kernel(x, w_q, b_q, w_k, b_k, w_v, b_v, w_o, b_o,
           a_q, u_q, a_k, u_k, a_v, u_v):
    cfg = Cfg()
    c = cfg
    inputs = {"x": x, "w_q": w_q, "w_k": w_k, "w_v": w_v, "w_o": w_o,
              "b_q": b_q, "b_k": b_k,
              "a_q": a_q, "u_q": u_q, "a_k": a_k, "u_k": u_k,
              "a_v": a_v, "u_v": u_v}
    in_maps = make_in_maps(inputs, c)
    nc = get_nc(cfg)
    res = run_bass_kernel_spmd(nc, in_maps, list(range(N_CORES)))
    out = np.zeros((c.seq, c.d), np.float32)
    for i in range(N_CORES):
        out += res.results[i]["out"]
    # v-bias rides through softmax as a constant row; b_o is plain bias
    b_v = np.asarray(b_v, np.float32)
    b_o = np.asarray(b_o, np.float32)
    out += (b_v @ np.asarray(w_o, np.float32) + b_o).astype(np.float32)
    return out.reshape(c.b, c.s, c.d).astype(np.float32)


# revision 67
# speedup vs baseline: 1.0080x; 1.0080x over previous
"""LoRA attention Bass kernel for 8x Trainium2 NeuronCores (bf16, pipelined).

Sharding (Megatron tensor-parallel over heads):
  - Each of the 8 cores owns 2 heads (128 projection columns).
  - q/k/v projections column-sharded; out projection row-sharded;
    per-core partial outputs are summed on the host.
  - LoRA is merged into the base weights on the host (w_eff = w + a@u*scaling),
    which is exact up to rounding.

Device kernel (per core), all matmuls in bf16 (fp32 PSUM accumulation):
  - Projections: qT/kT computed transposed ([proj_col, seq]) from xT tiles;
    v in natural [seq, col] layout from the same tiles.  Emitted in 256-wide
    half-chunks so q|k and v share one 2-bank PSUM slot.
  - Attention per (batch, 512-wide s-chunk): S^T = K @ Q^T per head into a
    shared [128, 2, 512] PSUM tile, ONE exp over both heads ([128,1024] ACT
    op), P@V with lhsT=[v | ones] so the softmax denominator falls out of the
    same matmul.  The PV matmuls trail the score matmuls by three t-steps so
    the PE never blocks on the ACT engine.
  - Normalization: reciprocal_approx_fast on the denominator row, K=1 matmul
    broadcast, fused out-projection, DMA to DRAM.
  - Software pipelining: the normalize+out-proj tail of chunk i and the
    projection work for batch b+1 are chopped into "pieces" and interleaved
    into chunk i+1's t-loop, filling PE idle slots while ACT (exp) runs
    back-to-back.  ACT is the critical path at ~1.06us per 1024-wide exp.

PSUM budget (8 banks): scores 2 slots x 2 banks + pv A/B 1 bank each +
shared proj/bc/out-proj slot 2 banks.
"""

import numpy as np
import ml_dtypes

import concourse.bass as bass
import concourse.mybir as mybir
import concourse.tile as tile
from concourse import bacc
from concourse.bass_utils import run_bass_kernel_spmd

F32 = mybir.dt.float32
F32R = mybir.dt.float32r
BF16 = mybir.dt.bfloat16
AF = mybir.ActivationFunctionType
MUL = mybir.AluOpType.mult

BF = ml_dtypes.bfloat16

N_CORES = 8

# Full-problem dims (hardcoded per spec)
D_MODEL = 1024
N_HEADS = 16
D_K = 64
LORA_R = 8
SCALING = 2.0
B = 4
S = 2048


class Cfg:
    """Kernel build configuration (parameterized so tests can build small)."""

    def __init__(self, b=B, s=S, d=D_MODEL, cpc=128, dk=D_K,
                 serial=False, debug=False):
        self.serial = serial            # no piece interleaving (bisect aid)
        self.debug = debug              # dump qT/kT/v to DRAM
        self.b = b                      # batches
        self.s = s                      # seq per batch
        self.d = d                      # model dim (projection contraction)
        self.cpc = cpc                  # projection cols per core (2 heads x 64)
        self.dk = dk                    # head dim
        self.seq = b * s                # total rows
        self.nkc = d // 128             # contraction chunks for projections
        self.sc = min(512, s)           # attention s-chunk width
        self.hw = self.sc // 2          # projection half-chunk width
        self.nsb = s // self.sc         # s-chunks per batch
        self.nt = s // 128              # t-chunks per batch
        self.nj = self.sc // 128        # 128-row groups per s-chunk
        self.ntt = self.seq // 128      # t-chunks total
        self.ne = max(1, d // self.sc)  # out-proj column groups
        assert self.ne <= 2 and self.hw % 128 == 0


def _build_nc(cfg: Cfg):
    c = cfg
    dk = c.dk
    nc = bacc.Bacc("TRN2", target_bir_lowering=False, debug=False,
                   num_devices=N_CORES)

    xT = nc.dram_tensor("xT", [c.d, c.seq], BF16, kind="ExternalInput").ap()
    wq = nc.dram_tensor("wq", [c.d, c.cpc], BF16, kind="ExternalInput").ap()
    wk = nc.dram_tensor("wk", [c.d, c.cpc], BF16, kind="ExternalInput").ap()
    wv = nc.dram_tensor("wv", [c.d, c.cpc], BF16, kind="ExternalInput").ap()
    wo = nc.dram_tensor("wo", [c.cpc, c.d], BF16, kind="ExternalInput").ap()
    bq = nc.dram_tensor("bq", [c.cpc, 1], F32, kind="ExternalInput").ap()
    bk = nc.dram_tensor("bk", [c.cpc, 1], F32, kind="ExternalInput").ap()
    ident = nc.dram_tensor("ident", [128, 128], BF16, kind="ExternalInput").ap()
    out = nc.dram_tensor("out", [c.seq, c.d], F32, kind="ExternalOutput").ap()
    if c.debug:
        qTd = nc.dram_tensor("qTd", [128, c.seq], BF16, kind="ExternalOutput").ap()
        kTd = nc.dram_tensor("kTd", [128, c.seq], BF16, kind="ExternalOutput").ap()
        vd = nc.dram_tensor("vd", [128, c.ntt, 2 * c.dk + 2], BF16,
                            kind="ExternalOutput").ap()
        scd = nc.dram_tensor("scd", [128, 2, c.sc], F32, kind="ExternalOutput").ap()
        ed = nc.dram_tensor("ed", [128, 2, c.sc], BF16, kind="ExternalOutput").ap()
        pvd = nc.dram_tensor("pvd", [c.dk + 1, 2, c.sc], F32,
                             kind="ExternalOutput").ap()
        recd = nc.dram_tensor("recd", [1, 2, c.sc], F32, kind="ExternalOutput").ap()
        normd = nc.dram_tensor("normd", [c.dk, 2, c.sc], BF16,
                               kind="ExternalOutput").ap()

    BK = 2048  # PSUM bank bytes

    def group_flags(items):
        """items: [(key, byte_offset)] of matmul writes into one PSUM slot.
        PSUM accumulation groups are per 2KB bank: return key -> (is_first,
        is_last) touch of its bank so start/stop is set exactly once per
        bank."""
        first, last = {}, {}
        for key, off in items:
            b = off // BK
            if b not in first:
                first[b] = key
            last[b] = key
        return {key: (first[off // BK] == key, last[off // BK] == key)
                for key, off in items}

    with tile.TileContext(nc) as tc:
        with tc.tile_pool(name="persist", bufs=1) as persist:
            qT_sb = persist.tile([128, c.seq], BF16, tag="qT")
            kT_sb = persist.tile([128, c.seq], BF16, tag="kT")
            # v natural + ones cols: [0:dk]=headA, dk=ones, [dk+1:2dk+1]=headB,
            # 2dk+1=ones
            v_sb = persist.tile([128, c.ntt, 2 * dk + 2], BF16, tag="v")
            wq_sb = persist.tile([128, c.nkc, c.cpc], BF16, tag="wq")
            wk_sb = persist.tile([128, c.nkc, c.cpc], BF16, tag="wk")
            wv_sb = persist.tile([128, c.nkc, c.cpc], BF16, tag="wv")
            wo_sb = persist.tile([c.cpc, c.d], BF16, tag="wo")
            bq_sb = persist.tile([c.cpc, 1], F32, tag="bq")
            bk_sb = persist.tile([c.cpc, 1], F32, tag="bk")
            id_sb = persist.tile([128, 128], BF16, tag="ident")
            nc.sync.dma_start(out=id_sb[:], in_=ident)

            nc.sync.dma_start(out=wq_sb[:], in_=wq.rearrange("(kc p) m -> p kc m", p=128))
            nc.sync.dma_start(out=wk_sb[:], in_=wk.rearrange("(kc p) m -> p kc m", p=128))
            nc.sync.dma_start(out=wv_sb[:], in_=wv.rearrange("(kc p) m -> p kc m", p=128))
            nc.sync.dma_start(out=wo_sb[:], in_=wo[:])
            nc.sync.dma_start(out=bq_sb[:], in_=bq[:])
            nc.sync.dma_start(out=bk_sb[:], in_=bk[:])

            ones_f32 = persist.tile([128, 1], F32, tag="ones_f32")
            nc.vector.memset(ones_f32[:], 1.0)
            # ones columns for the fused softmax denominator
            nc.vector.tensor_copy(
                v_sb[:, :, dk:dk + 1],
                ones_f32[:].unsqueeze(1).to_broadcast([128, c.ntt, 1]))
            nc.vector.tensor_copy(
                v_sb[:, :, 2 * dk + 1:2 * dk + 2],
                ones_f32[:].unsqueeze(1).to_broadcast([128, c.ntt, 1]))
            # ones row at partition dk (bf16, for the bf16 bc matmul)
            ones_bc = persist.tile([dk + 1, dk], BF16, tag="ones_bc")
            nc.vector.tensor_copy(
                ones_bc[:], ones_f32[0:dk + 1, :].to_broadcast([dk + 1, dk]))
            # zero-weights row + defined rhs row for PSUM bank-clearing
            # matmuls: on HW has_written bits persist across accumulation
            # groups, so every group must open with a start=True matmul that
            # covers the ENTIRE bank it accumulates into.
            zrow_sb = persist.tile([1, 128], BF16, tag="zrow")
            nc.vector.memset(zrow_sb[:], 0.0)
            orow_sb = persist.tile([1, 512], BF16, tag="orow")
            nc.vector.memset(orow_sb[:], 1.0)

            with tc.tile_pool(name="xp", bufs=3) as xpool, \
                 tc.tile_pool(name="ep", bufs=7) as epool, \
                 tc.tile_pool(name="np", bufs=3) as npool, \
                 tc.tile_pool(name="rp", bufs=3) as rpool, \
                 tc.tile_pool(name="bp", bufs=3) as bpool, \
                 tc.tile_pool(name="op", bufs=4) as opool, \
                 tc.tile_pool(name="scp", bufs=2, space="PSUM") as scp, \
                 tc.tile_pool(name="pvp", bufs=1, space="PSUM") as pvp, \
                 tc.tile_pool(name="wkp", bufs=1, space="PSUM") as wkp:

                def work_tile(name):
                    # shared 2-bank PSUM slot for proj / bc / out-proj
                    return wkp.tile([128, 2, c.sc], F32, tag="wk", name=name)

                def proj_pieces(b, pc):
                    """Piece closures computing q/k/vT for s-chunk pc of
                    batch b with full-width (512) matmuls: q -> work bank0,
                    k -> bank1, then vT reuses the slot; vT is converted to
                    natural layout by PE transposes (identity rhs)."""
                    pieces = []
                    s0 = b * c.s + pc * c.sc
                    tc0 = s0 // 128
                    qfl = group_flags([("q", 0), ("k", c.sc * 4)])
                    PF = 4  # x-tile DMA prefetch depth (pieces ahead)
                    xts = {}
                    st8 = {}

                    def fetch_x(kc):
                        x_t = xpool.tile([128, c.sc], BF16, tag="xf", bufs=8,
                                         name=f"x_{b}_{pc}_{kc}")
                        nc.sync.dma_start(
                            out=x_t[:],
                            in_=xT[kc * 128:(kc + 1) * 128, s0:s0 + c.sc])
                        xts[kc] = x_t

                    for kc in range(c.nkc):
                        def fq(kc=kc):
                            if kc == 0:
                                st8["qk"] = work_tile(f"pqk_{b}_{pc}")
                                for p in range(min(PF + 1, c.nkc)):
                                    fetch_x(p)
                            elif kc + PF < c.nkc:
                                fetch_x(kc + PF)
                            qk = st8["qk"]
                            nc.tensor.matmul(qk[:, 0, :], wq_sb[:, kc, :],
                                             xts[kc][:],
                                             start=(kc == 0) and qfl["q"][0],
                                             stop=(kc == c.nkc - 1) and qfl["q"][1])
                        pieces.append(fq)
                    for kc in range(c.nkc):
                        def fk(kc=kc):
                            qk = st8["qk"]
                            nc.tensor.matmul(qk[:, 1, :], wk_sb[:, kc, :],
                                             xts[kc][:],
                                             start=(kc == 0) and qfl["k"][0],
                                             stop=(kc == c.nkc - 1) and qfl["k"][1])
                            if kc == c.nkc - 1:
                                nc.vector.tensor_scalar_add(
                                    qT_sb[:, s0:s0 + c.sc], qk[:, 0, :],
                                    bq_sb[:])
                                nc.vector.tensor_scalar_add(
                                    kT_sb[:, s0:s0 + c.sc], qk[:, 1, :],
                                    bk_sb[:])
                        pieces.append(fk)
                    for kc in range(c.nkc):
                        def fv(kc=kc):
                            if kc == 0:
                                st8["vt"] = work_tile(f"pvt_{b}_{pc}")
                            vt = st8["vt"]
                            nc.tensor.matmul(vt[:, 0, :], wv_sb[:, kc, :],
                                             xts[kc][:],
                                             start=(kc == 0),
                                             stop=(kc == c.nkc - 1))
                            if kc == c.nkc - 1:
                                vts = bpool.tile([128, c.sc], BF16, tag="vts",
                                                 name=f"vts_{b}_{pc}")
                                nc.vector.tensor_copy(vts[:], vt[:, 0, :])
                                st8["vts"] = vts
                        pieces.append(fv)
                    for j in range(c.nj):
                        def ftr(j=j):
                            vts = st8["vts"]
                            tp = work_tile(f"vtr_{b}_{pc}_{j}")
                            tb = tp[:, 1, :].bitcast(BF16)
                            nc.tensor.matmul(tb[:, 0:128],
                                             vts[:, j * 128:(j + 1) * 128],
                                             id_sb[:], is_transpose=True,
                                             start=True, stop=True)
                            dst = v_sb[:, tc0 + j, :].rearrange(
                                "p (h x) -> p h x", x=dk + 1)[:, :, 0:dk]
                            src = tb[:, 0:128].rearrange(
                                "p (h x) -> p h x", x=dk)
                            nc.vector.tensor_copy(dst, src)
                        pieces.append(ftr)
                    return pieces

                def emit_proj_full(b, pc):
                    """Prologue-only projection: full 512-wide q/k matmuls
                    using a free scores-pool slot (t-loops haven't started)."""
                    s0 = b * c.s + pc * c.sc
                    tc0 = s0 // 128
                    qk = scp.tile([128, 2, c.sc], F32, tag="sc",
                                  name=f"pqk_{b}_{pc}")
                    vw = work_tile(f"pv_{b}_{pc}")
                    nc.tensor.matmul(vw[:, 1, :], zrow_sb[:], orow_sb[:, 0:c.sc],
                                     start=True, stop=False)
                    xts = {}

                    def fetch(kc):
                        x_t = xpool.tile([128, c.sc], BF16, tag="xf",
                                         name=f"xf_{b}_{pc}_{kc}", bufs=8)
                        nc.sync.dma_start(
                            out=x_t[:],
                            in_=xT[kc * 128:(kc + 1) * 128, s0:s0 + c.sc])
                        xts[kc] = x_t

                    for p in range(min(6, c.nkc)):
                        fetch(p)
                    for kc in range(c.nkc):
                        if kc + 6 < c.nkc:
                            fetch(kc + 6)
                        x_t = xts.pop(kc)
                        k0, kN = kc == 0, kc == c.nkc - 1
                        qfl = group_flags([("q", 0), ("k", c.sc * 4)])
                        nc.tensor.matmul(qk[:, 0, :], wq_sb[:, kc, :], x_t[:],
                                         start=k0 and qfl["q"][0],
                                         stop=kN and qfl["q"][1])
                        nc.tensor.matmul(qk[:, 1, :], wk_sb[:, kc, :], x_t[:],
                                         start=k0 and qfl["k"][0],
                                         stop=kN and qfl["k"][1])
                        for j in range(c.nj):
                            nc.tensor.matmul(vw[:, 1, j * 128:(j + 1) * 128],
                                             x_t[:, j * 128:(j + 1) * 128],
                                             wv_sb[:, kc, :],
                                             start=False, stop=kN and j == c.nj - 1)
                    nc.vector.tensor_scalar_add(qT_sb[:, s0:s0 + c.sc],
                                                qk[:, 0, :], bq_sb[:])
                    nc.vector.tensor_scalar_add(kT_sb[:, s0:s0 + c.sc],
                                                qk[:, 1, :], bk_sb[:])
                    v4 = vw[:, 1, :].rearrange("p (j x) -> p j x", x=128)
                    nc.vector.tensor_copy(v_sb[:, tc0:tc0 + c.nj, 0:dk],
                                          v4[:, :, 0:dk])
                    nc.vector.tensor_copy(
                        v_sb[:, tc0:tc0 + c.nj, dk + 1:2 * dk + 1],
                        v4[:, :, dk:2 * dk])

                def tail_pieces(b, sb, pva, pvb):
                    """Normalize + out-project chunk (b, sb); emitted one
                    chunk late so the work interleaves into the next t-loop."""
                    s0g = b * c.s + sb * c.sc
                    # Free the PV PSUM banks ASAP: copy the full accumulators
                    # to SBUF right away; everything downstream reads the
                    # copy.  (pvp bufs=1 — the next chunk's PV matmuls wait
                    # only on this copy, not on the whole normalize chain.)
                    pvs = rpool.tile([dk + 1, 2, c.sc], F32, tag="pvs",
                                     name=f"pvs_{b}_{sb}")
                    nc.vector.tensor_copy(pvs[:, 0, :], pva[:])
                    nc.vector.tensor_copy(pvs[:, 1, :], pvb[:])
                    # denominator rows as bf16 for the broadcast matmul
                    # (reciprocal_approx_fast is broken for single-row APs at
                    # partition 64 on HW, so broadcast the DENOMINATOR first,
                    # then recip on the 64-partition broadcast tile)
                    denb = rpool.tile([dk + 1, 2, c.sc], BF16, tag="denb",
                                      name=f"denb_{b}_{sb}")
                    nc.vector.tensor_copy(denb[dk:dk + 1, :, :],
                                          pvs[dk:dk + 1, :, :])
                    if c.debug and b == 0 and sb == 0:
                        nc.sync.dma_start(out=pvd, in_=pvs[:])
                    st8 = {}
                    pieces = []

                    def bc_f():
                        bfl = group_flags([("a", 0), ("b", c.sc * 4)])
                        bcp = work_tile(f"bc_{b}_{sb}")
                        nc.tensor.matmul(bcp[0:dk, 0, :],
                                         ones_bc[dk:dk + 1, :],
                                         denb[dk:dk + 1, 0, :],
                                         start=bfl["a"][0], stop=bfl["a"][1])
                        nc.tensor.matmul(bcp[0:dk, 1, :],
                                         ones_bc[dk:dk + 1, :],
                                         denb[dk:dk + 1, 1, :],
                                         start=bfl["b"][0], stop=bfl["b"][1])
                        bcs = bpool.tile([dk, 2, c.sc], F32, tag="bcs",
                                         name=f"bcs_{b}_{sb}")
                        bcr = bpool.tile([dk, 2, c.sc], F32, tag="bcr",
                                         name=f"bcr_{b}_{sb}")
                        norm = npool.tile([dk, 2, c.sc], BF16, tag="norm",
                                          name=f"norm_{b}_{sb}")
                        nc.vector.tensor_copy(bcs[:], bcp[0:dk, :, :])
                        nc.vector.reciprocal_approx_fast(out=bcr[:], in_=bcs[:])
                        nc.vector.tensor_tensor(norm[:, 0, :], pvs[0:dk, 0, :],
                                                bcr[:, 0, :], MUL)
                        nc.vector.tensor_tensor(norm[:, 1, :], pvs[0:dk, 1, :],
                                                bcr[:, 1, :], MUL)
                        if c.debug and b == 0 and sb == 0:
                            nc.sync.dma_start(out=recd, in_=bcr[0:1, :, :])
                            nc.sync.dma_start(out=normd, in_=norm[:])
                        st8["norm"] = norm
                    pieces.append(bc_f)

                    def asm_f():
                        # stack the two heads' normalized attention into one
                        # [128, sc] tile via K=64 identity matmuls (headB
                        # lands at partitions 64-127), so each out-proj group
                        # is a single K=128 matmul against unsliced w_o
                        norm = st8["norm"]
                        aps = work_tile(f"asm_{b}_{sb}")
                        nc.tensor.matmul(aps[0:dk, 0, :],
                                         id_sb[0:dk, 0:dk], norm[:, 0, :],
                                         start=True, stop=True)
                        nc.tensor.matmul(aps[dk:2 * dk, 0, :],
                                         id_sb[0:dk, 0:dk], norm[:, 1, :],
                                         start=True, stop=True)
                        n2 = npool.tile([128, c.sc], BF16, tag="n2",
                                        name=f"n2_{b}_{sb}")
                        nc.vector.tensor_copy(n2[:], aps[:, 0, :])
                        st8["n2"] = n2
                    pieces.append(asm_f)

                    for j in range(c.nj):
                        def o_f(j=j):
                            n2 = st8["n2"]
                            ops = work_tile(f"ops_{b}_{sb}_{j}")
                            for ei in range(c.ne):
                                nc.tensor.matmul(
                                    ops[:, ei, 0:min(c.sc, c.d)],
                                    n2[:, j * 128:(j + 1) * 128],
                                    wo_sb[:, ei * c.sc:min((ei + 1) * c.sc, c.d)],
                                    start=True, stop=True)
                            o_t = opool.tile([128, c.d], F32, tag="o",
                                             name=f"ot_{b}_{sb}_{j}")
                            ov = o_t.rearrange("p (e x) -> p e x",
                                               x=min(c.sc, c.d))
                            nc.vector.tensor_copy(
                                ov[:, :, :], ops[:, 0:c.ne, 0:min(c.sc, c.d)])
                            nc.sync.dma_start(
                                out=out[s0g + j * 128:s0g + (j + 1) * 128, :],
                                in_=o_t[:])
                        pieces.append(o_f)
                    return pieces

                def emit_tloop(b, sb, pieces):
                    s0 = b * c.s + sb * c.sc
                    pva = pvp.tile([dk + 1, c.sc], F32, tag="pva",
                                   name=f"pva_{b}_{sb}")
                    pvb = pvp.tile([dk + 1, c.sc], F32, tag="pvb",
                                   name=f"pvb_{b}_{sb}")
                    pend = []
                    npc = len(pieces)
                    pi = 0
                    lag = min(5, c.nt)  # pv trails exp so PE never waits ACT
                    sfl = group_flags([("a", 0), ("b", c.sc * 4)])

                    def emit_pv(pt, pe_t):
                        tci = b * c.nt + pt
                        nc.tensor.matmul(pva[:], v_sb[:, tci, 0:dk + 1],
                                         pe_t[:, 0, :], start=(pt == 0),
                                         stop=(pt == c.nt - 1))
                        nc.tensor.matmul(pvb[:],
                                         v_sb[:, tci, dk + 1:2 * dk + 2],
                                         pe_t[:, 1, :], start=(pt == 0),
                                         stop=(pt == c.nt - 1))

                    for t in range(c.nt):
                        t0 = b * c.s + t * 128
                        scs = scp.tile([128, 2, c.sc], F32, tag="sc",
                                       name=f"scs_{b}_{sb}_{t}")
                        nc.tensor.matmul(scs[:, 0, :],
                                         kT_sb[0:dk, t0:t0 + 128],
                                         qT_sb[0:dk, s0:s0 + c.sc],
                                         start=sfl["a"][0], stop=sfl["a"][1])
                        nc.tensor.matmul(scs[:, 1, :],
                                         kT_sb[dk:2 * dk, t0:t0 + 128],
                                         qT_sb[dk:2 * dk, s0:s0 + c.sc],
                                         start=sfl["b"][0], stop=sfl["b"][1])
                        e_t = epool.tile([128, 2, c.sc], BF16, tag="e",
                                         name=f"e_{b}_{sb}_{t}")
                        if c.debug and b == 0 and sb == 0 and t == 0:
                            scs_d = opool.tile([128, 2, c.sc], F32, tag="scd",
                                               name="scs_dump")
                            nc.vector.tensor_copy(scs_d[:], scs[:])
                            nc.sync.dma_start(out=scd, in_=scs_d[:])
                        nc.scalar.activation(e_t[:], scs[:], AF.Exp,
                                             scale=1.0 / np.sqrt(dk))
                        if c.debug and b == 0 and sb == 0 and t == 0:
                            nc.sync.dma_start(out=ed, in_=e_t[:])
                        want = max((npc * (t + 1)) // c.nt, min(npc, 1))
                        while pi < want:
                            pieces[pi]()
                            pi += 1
                        pend.append((t, e_t))
                        if len(pend) >= lag:
                            emit_pv(*pend.pop(0))
                    for pt, pe_t in pend:
                        emit_pv(pt, pe_t)
                    return pva, pvb

                # ---- emission: short prologue, proj queue, pipelined chunks
                # Prologue covers proj(0, 0..nsb-2) serially; the remaining
                # projection chunks stream through a queue, one per attention
                # chunk (two for chunk 0, which has no tail pieces).  Batch
                # b+1's projections all land before attention chunk (b+1, 0).
                npro = max(c.nsb - 2, 1)
                for pc in range(npro):
                    emit_proj_full(0, pc)
                pq = [(0, pc) for pc in range(npro, c.nsb)]
                pq += [(b + 1, sb) for b in range(c.b - 1)
                       for sb in range(c.nsb)]
                prev = None
                ci = 0
                for b in range(c.b):
                    for sb in range(c.nsb):
                        pieces = []
                        if prev is not None:
                            pieces += tail_pieces(*prev)
                        pops = 3 if ci == 0 else 1
                        for _ in range(pops):
                            if pq:
                                pieces += proj_pieces(*pq.pop(0))
                        if c.serial:
                            for f in pieces:
                                f()
                            pieces = []
                        pva, pvb = emit_tloop(b, sb, pieces)
                        prev = (b, sb, pva, pvb)
                        ci += 1
                for f in tail_pieces(*prev):
                    f()
                if c.debug:
                    nc.sync.dma_start(out=qTd, in_=qT_sb[:])
                    nc.sync.dma_start(out=kTd, in_=kT_sb[:])
                    nc.sync.dma_start(out=vd, in_=v_sb[:])

    nc.compile()
    return nc


_NC_CACHE = {}


def get_nc(cfg: Cfg | None = None):
    cfg = cfg or Cfg()
    key = (cfg.b, cfg.s, cfg.d, cfg.cpc, cfg.dk, cfg.serial, cfg.debug)
    if key not in _NC_CACHE:
        _NC_CACHE[key] = _build_nc(cfg)
    return _NC_CACHE[key]


def _merge(w, a, u):
    return (np.asarray(w, np.float64)
            + (np.asarray(a, np.float64) @ np.asarray(u, np.float64))
            * SCALING)


def make_in_maps(inputs, cfg: Cfg | None = None):
    """Host-side prep: merge LoRA, transpose x, cast to bf16, shard by head."""
    c = cfg or Cfg()
    x = np.asarray(inputs["x"], np.float32)
    wq_eff = _merge(inputs["w_q"], inputs["a_q"], inputs["u_q"])
    wk_eff = _merge(inputs["w_k"], inputs["a_k"], inputs["u_k"])
    wv_eff = _merge(inputs["w_v"], inputs["a_v"], inputs["u_v"])
    w_o = np.asarray(inputs["w_o"], np.float32)
    b_q = np.asarray(inputs["b_q"], np.float32)
    b_k = np.asarray(inputs["b_k"], np.float32)

    xT = np.ascontiguousarray(x.reshape(c.seq, c.d).T.astype(BF))
    ident = np.eye(128, dtype=BF)
    in_maps = []
    for i in range(N_CORES):
        sl = slice(i * c.cpc, (i + 1) * c.cpc)
        in_maps.append({
            "xT": xT,
            "ident": ident,
            "wq": np.ascontiguousarray(wq_eff[:, sl].astype(BF)),
            "wk": np.ascontiguousarray(wk_eff[:, sl].astype(BF)),
            "wv": np.ascontiguousarray(wv_eff[:, sl].astype(BF)),
            "wo": np.ascontiguousarray(w_o[sl, :].astype(BF)),
            "bq": np.ascontiguousarray(b_q[sl]).reshape(c.cpc, 1),
            "bk": np.ascontiguousarray(b_k[sl]).reshape(c.cpc, 1),
        })
    return in_maps


def kernel(x, w_q, b_q, w_k, b_k, w_v, b_v, w_o, b_o,
           a_q, u_q, a_k, u_k, a_v, u_v):
    cfg = Cfg()
    c = cfg
    inputs = {"x": x, "w_q": w_q, "w_k": w_k, "w_v": w_v, "w_o": w_o,
              "b_q": b_q, "b_k": b_k,
              "a_q": a_q, "u_q": u_q, "a_k": a_k, "u_k": u_k,
              "a_v": a_v, "u_v": u_v}
    in_maps = make_in_maps(inputs, c)
    nc = get_nc(cfg)
    res = run_bass_kernel_spmd(nc, in_maps, list(range(N_CORES)))
    out = np.zeros((c.seq, c.d), np.float32)
    for i in range(N_CORES):
        out += res.results[i]["out"]
    # v-bias rides through softmax as a constant row; b_o is plain bias
    b_v = np.asarray(b_v, np.float32)
    b_o = np.asarray(b_o, np.float32)
    out += (b_v @ np.asarray(w_o, np.float32) + b_o).astype(np.float32)
    return out.reshape(c.b, c.s, c.d).astype(np.float32)
